# revision 35
# baseline (speedup 1.0000x reference)
"""Multi-head attention (B=2, S=2048, D=1024, H=16, hd=64, RoPE, causal)
on 8 Trainium2 NeuronCores.

Sharding: each core owns 2 heads x both batches (tensor-parallel over heads)
through attention; the out-projection is position-sharded (each core owns a
disjoint slice of positions) via five pipelined small AllToAlls that fire as
each half-batch / chunk of attention output becomes ready, so only the last
~128KB collective is exposed at the tail.

Per core, feature-major [feature, seq] layout, all matmuls bf16:
  - inputs arrive as host-preswizzled contiguous 1MB blocks, streamed on
    BOTH HWDGE rings (sync + scalar) with the first weight/x slices split
    fine so the first matmul starts ~10us in
  - Q/K/V projections per 512-chunk; bias via DVE tensor_scalar; RoPE via a
    PE permutation matmul + 3 DVE bf16 ops; V is PE-transposed into a
    key-major aggregate with a ones-column appended per head (the softmax
    denominator falls out of the attnV matmul for free)
  - scores TRANSPOSED ST[k,q] = KT_h.T @ QT_h so no softmax max-subtraction
    is needed (scores bounded); causal at 128-tile granularity; key-tiles
    width-packed into 2-bank PSUM groups so exp runs as few wide ACT calls;
    the triangular mask is added to both diagonal tiles of a group in one
    strided DVE op
  - the two heads' score matmuls are interleaved per group: they use
    disjoint PE row groups (partitions 0-63 / 64-127), which lets the PE's
    in-silicon LDWEIGHTS pull-ahead overlap weight loads with matmuls (the
    walrus build here has ldw double-buffering disabled, so this is the only
    way to dodge the ~350-cycle per-matmul LDW+drain tax)
  - softmax normalization: denominator reciprocal via ACT Ln/Exp (same
    table set as the score exp), broadcast across partitions by a rank-1 PE
    matmul, applied by DVE
  - next-chunk projection matmuls are zippered into the current chunk's
    attention instruction stream (one emitter per exp group), filling the
    PE bubbles where attnV waits on ACT; the final out-projections ride the
    last attention chunk the same way
  - out-projection consumes the AllToAll'd [1024 feat x pos] slabs and
    produces y[pos, D] directly (positions on partitions), bias added by
    DVE from a partition-broadcast bias tile
"""
import os

import ml_dtypes
import numpy as np

import concourse.bass as bass
import concourse.mybir as mybir
import concourse.tile as tile
from concourse.bass_utils import run_bass_kernel_spmd
from concourse.vector_clock import ScopedClock

B, S, D, H, HD = 2, 2048, 1024, 16, 64
NCORES = 8
HPC = 2                    # heads per core
F = HPC * HD               # 128 features per core
CHUNK = 512
NCH = S // CHUNK           # 4 q-chunks per batch
NKT = D // 128             # 8 contraction tiles for projections
NST = S // 128             # 16 key tiles
NH = 4                     # half-batches (b, half) = collective units
MASKVAL = -240.0           # -30 after the 1/8 softmax scale
F32 = mybir.dt.float32
F32R = mybir.dt.float32r
BF16 = mybir.dt.bfloat16
BF = ml_dtypes.bfloat16
AF = mybir.ActivationFunctionType


# ---------------------------------------------------------------------------
# Workarounds for the walrus build in this container: it encodes at most ONE
# sync-wait per instruction ("Too many sync wait commands"). Split multi-wait
# instructions into single-wait NoOps. Semantics-preserving.
# ---------------------------------------------------------------------------
_patched = False


def _install_patches():
    global _patched
    if _patched:
        return
    _patched = True

    _orig_lower = tile.TileContext._lower_ordered_insts

    def _lower_with_wait_split(self, ordered):
        nc = self.nc
        for _bb, insts in ordered.items():
            if not any(
                i.sync_info is not None and len(i.sync_info.on_wait) > 1
                for i in insts
            ):
                continue
            new = []
            for inst in insts:
                si = inst.sync_info
                if si is not None and len(si.on_wait) > 1:
                    waits = list(si.on_wait)
                    for w in waits[:-1]:
                        n = mybir.InstNoOp(
                            name=f"I-waitsplit-{nc.next_id()}", ins=[], outs=[]
                        )
                        n.engine = inst.engine
                        n.bass_nofuse = True
                        n.sync_info = mybir.SyncInfo(on_wait=[w], on_update=[])
                        nc.register_instruction(n)
                        new.append(n)
                    inst.sync_info = mybir.SyncInfo(
                        on_wait=[waits[-1]], on_update=list(si.on_update)
                    )
                new.append(inst)
            insts[:] = new
        return _orig_lower(self, ordered)

    tile.TileContext._lower_ordered_insts = _lower_with_wait_split

    def _drain_and_barrier(self, tick_clock, wait_clock):
        nc = self.nc
        probe = nc.sync.nop(nofuse=True)
        wait_clock.add_sem_waits(
            probe.ins, ScopedClock({None: tick_clock.global_clock})
        )
        waits = list(probe.ins.sync_info.on_wait)
        probe.ins.sync_info = mybir.SyncInfo(on_wait=waits[:1], on_update=[])
        for w in waits[1:]:
            n2 = nc.sync.nop(nofuse=True)
            n2.ins.sync_info = mybir.SyncInfo(on_wait=[w], on_update=[])
        nc.sync.drain()
        nc.all_engine_barrier()
        assert self.sems is not None
        popped = nc._tile_sem_poison_stack.pop()
        assert popped is self._sem_poison
        nc.clear_and_free_semaphores(list(self.sems.allocated().values()))
        nc.all_engine_barrier()

    tile.TileContext._drain_and_barrier = _drain_and_barrier


def _install_ntff_hook():
    """Provide the missing ``antenv.axon_hooks`` module so trace=True works."""
    import sys
    import types

    if "antenv.axon_hooks" in sys.modules:
        return
    try:
        import antenv
        from trn_agent_boot.trn_boot import _ntff_profile_via_ctypes
    except ImportError:
        return
    mod = types.ModuleType("antenv.axon_hooks")
    mod._hook = _ntff_profile_via_ctypes("/opt/axon/libaxon_pjrt.so")
    mod.set_axon_ntff_profile_hook = lambda h: setattr(mod, "_hook", h)
    mod.get_axon_ntff_profile_hook = lambda: mod._hook
    sys.modules["antenv.axon_hooks"] = mod
    antenv.axon_hooks = mod


def _score_groups(c):
    """Pack the causal key-tiles of q-chunk c into PSUM groups of <=1024
    columns. Returns [[(kt, qlo, w, off), ...], ...]."""
    groups, cur, cw = [], [], 0
    for kt in range(4 * c + 4):
        qlo = max(CHUNK * c, 128 * kt)
        w = CHUNK * (c + 1) - qlo
        if cw + w > 1024:
            groups.append(cur)
            cur, cw = [], 0
        # matmul output must not straddle a PSUM bank boundary
        assert cw % 512 == 0 or cw + w <= 512, (c, kt, cw, w)
        cur.append((kt, qlo, w, cw))
        cw += w
    if cur:
        groups.append(cur)
    return groups


# ---------------------------------------------------------------------------
# Program builder (same program on all 8 cores; per-core data differs)
# ---------------------------------------------------------------------------
def build_program():
    _install_patches()
    nc = bass.Bass(num_devices=NCORES)

    xtc_d = nc.dram_tensor("xtc", [B * NCH, 128, NKT * CHUNK], BF16,
                           kind="ExternalInput")
    wq_d = nc.dram_tensor("wq", [128, NKT * F], BF16, kind="ExternalInput")
    wk_d = nc.dram_tensor("wk", [128, NKT * F], BF16, kind="ExternalInput")
    wv_d = nc.dram_tensor("wv", [128, NKT * F], BF16, kind="ExternalInput")
    bq_d = nc.dram_tensor("bq", [F, 1], F32, kind="ExternalInput")
    bk_d = nc.dram_tensor("bk", [F, 1], F32, kind="ExternalInput")
    bv_d = nc.dram_tensor("bv", [F, 1], F32, kind="ExternalInput")
    ident_d = nc.dram_tensor("ident128", [128, 128], BF16,
                             kind="ExternalInput")
    chat_d = nc.dram_tensor("chat", [128, S], BF16, kind="ExternalInput")
    shat_d = nc.dram_tensor("shat", [128, S], BF16, kind="ExternalInput")
    mask_d = nc.dram_tensor("mask128", [128, 128], F32, kind="ExternalInput")
    perm_d = nc.dram_tensor("perm128", [128, 128], BF16, kind="ExternalInput")
    ones64_d = nc.dram_tensor("ones64", [1, 64], BF16, kind="ExternalInput")
    borow_d = nc.dram_tensor("borow", [1, D], BF16, kind="ExternalInput")
    wot_d = nc.dram_tensor("wot", [128, NKT * D], BF16, kind="ExternalInput")
    ytq = nc.dram_tensor("ytq", [NH, 128, D], F32, kind="ExternalOutput")

    debug = bool(int(os.environ.get("MHA_DEBUG", "0")))
    if debug:
        dbg_qt = nc.dram_tensor("dbg_qt", [F, S], BF16, kind="ExternalOutput")
        dbg_kt = nc.dram_tensor("dbg_kt", [F, S], BF16, kind="ExternalOutput")
        dbg_vagg = nc.dram_tensor("dbg_vagg", [128, NST * HPC * 65], BF16,
                                  kind="ExternalOutput")
        dbg_sg = nc.dram_tensor("dbg_sg", [128, CHUNK], BF16,
                                kind="ExternalOutput")
        dbg_a2ao = nc.dram_tensor("dbg_a2ao", [NCORES, F * 128], BF16,
                                  kind="ExternalOutput")
        dbg_a2ai = nc.dram_tensor("dbg_a2ai", [NCORES, F * 128], BF16,
                                  kind="ExternalOutput")

    a2a_in = [nc.dram_tensor(f"a2ain{hh}", [NCORES, F * 128], BF16)
              for hh in range(3)]
    a2a_out = [nc.dram_tensor(f"a2aout{hh}", [NCORES, F * 128], BF16)
               for hh in range(3)]
    a2a_in3 = [nc.dram_tensor(f"a2ain3{j}", [NCORES, F * 64], BF16)
               for j in range(2)]
    a2a_out3 = [nc.dram_tensor(f"a2aout3{j}", [NCORES, F * 64], BF16)
                for j in range(2)]

    with tile.TileContext(nc) as tc:
        with (
            tc.tile_pool(name="const", bufs=1) as const,
            tc.tile_pool(name="xtp", bufs=1) as xtp,
            tc.tile_pool(name="qkv", bufs=2) as qkv,
            tc.tile_pool(name="vaggp", bufs=2) as vaggp,
            tc.tile_pool(name="rawp", bufs=3) as rawp,
            tc.tile_pool(name="ropet", bufs=3) as ropet,
            tc.tile_pool(name="exp", bufs=6) as expp,
            tc.tile_pool(name="recp", bufs=4) as recp,
            tc.tile_pool(name="sgp", bufs=3) as sgp,
            tc.tile_pool(name="at2p", bufs=2) as at2p,
            tc.tile_pool(name="ysp", bufs=2) as ysp,
            tc.tile_pool(name="psm", bufs=2, space="PSUM") as psm,
            tc.tile_pool(name="pssc", bufs=2, space="PSUM") as pssc,
            tc.tile_pool(name="psav", bufs=2, space="PSUM") as psav,
        ):
            # ---- input DMAs, priority order on the sync ring ----
            xt_t = {}
            # ramp-critical path on the sync ring: wq + first x chunk only;
            # everything else for chunk 0/1 goes to the scalar ring so both
            # HWDGE rings stream in parallel.
            wq_t = const.tile([128, NKT * F], BF16, name="wq")
            nc.sync.dma_start(out=wq_t[:, 0:F], in_=wq_d[:, 0:F])
            bq_t = const.tile([F, 1], F32, name="bq")
            nc.sync.dma_start(out=bq_t, in_=bq_d[:])
            t00 = xtp.tile([128, NKT * CHUNK], BF16, tag="xt00", name="xt00")
            nc.sync.dma_start(out=t00[:, 0:CHUNK], in_=xtc_d[0][:, 0:CHUNK])
            nc.sync.dma_start(out=wq_t[:, F:2 * F], in_=wq_d[:, F:2 * F])
            nc.sync.dma_start(out=t00[:, CHUNK:2 * CHUNK],
                              in_=xtc_d[0][:, CHUNK:2 * CHUNK])
            nc.sync.dma_start(out=wq_t[:, 2 * F:], in_=wq_d[:, 2 * F:])
            nc.sync.dma_start(out=t00[:, 2 * CHUNK:4 * CHUNK],
                              in_=xtc_d[0][:, 2 * CHUNK:4 * CHUNK])
            nc.sync.dma_start(out=t00[:, 4 * CHUNK:],
                              in_=xtc_d[0][:, 4 * CHUNK:])
            xt_t[0, 0] = t00
            wk_t = const.tile([128, NKT * F], BF16, name="wk")
            wv_t = const.tile([128, NKT * F], BF16, name="wv")
            nc.scalar.dma_start(out=wk_t, in_=wk_d[:])
            nc.scalar.dma_start(out=wv_t, in_=wv_d[:])
            bk_t = const.tile([F, 1], F32, name="bk")
            bv_t = const.tile([F, 1], F32, name="bv")
            ident_t = const.tile([128, 128], BF16, name="ident")
            nc.scalar.dma_start(out=bk_t, in_=bk_d[:])
            nc.scalar.dma_start(out=bv_t, in_=bv_d[:])
            nc.scalar.dma_start(out=ident_t, in_=ident_d[:])
            chat_t = const.tile([128, S], BF16, name="chat")
            shat_t = const.tile([128, S], BF16, name="shat")
            nc.scalar.dma_start(out=chat_t, in_=chat_d[:])
            nc.scalar.dma_start(out=shat_t, in_=shat_d[:])
            mask_t = const.tile([128, 128], F32, name="mask")
            perm_t = const.tile([128, 128], BF16, name="perm")
            ones64_t = const.tile([1, 64], BF16, name="ones64")
            borowb_t = const.tile([128, D], BF16, name="borowb")
            nc.scalar.dma_start(out=mask_t, in_=mask_d[:])
            nc.scalar.dma_start(out=perm_t, in_=perm_d[:])
            nc.scalar.dma_start(out=ones64_t, in_=ones64_d[:])
            _br = borow_d[:]
            nc.scalar.dma_start(
                out=borowb_t,
                in_=bass.AP(tensor=_br.tensor, offset=_br.offset,
                            ap=[[0, 128]] + [list(p) for p in _br.ap[1:]]),
            )
            t01 = xtp.tile([128, NKT * CHUNK], BF16, tag="xt01", name="xt01")
            nc.scalar.dma_start(out=t01, in_=xtc_d[1])
            xt_t[0, 1] = t01
            for b in range(B):
                for c in range(NCH):
                    if (b, c) in ((0, 0), (0, 1)):
                        continue
                    t = xtp.tile([128, NKT * CHUNK], BF16, tag=f"xt{b}{c}",
                                 name=f"xt{b}{c}")
                    nc.sync.dma_start(out=t, in_=xtc_d[NCH * b + c])
                    xt_t[b, c] = t
            # out-proj weights on the scalar ring (idle early)
            wot_t = const.tile([128, NKT * D], BF16, name="wot")
            nc.scalar.dma_start(out=wot_t, in_=wot_d[:])

            state = {}

            def proj_emitters(b, c):
                """Fine-grained emitter closures for chunk (b, c)'s
                projections, to zipper into the previous chunk's attention."""
                env = {}

                def setup():
                    if c == 0:
                        state["QT"] = qkv.tile([F, S], BF16, tag="QT",
                                               name="QT")
                        state["KT"] = qkv.tile([F, S], BF16, tag="KT",
                                               name="KT")
                        state["VT"] = qkv.tile([F, S], BF16, tag="VT",
                                               name="VT")
                        vagg = vaggp.tile([128, NST * HPC * 65], BF16,
                                          tag="vagg", name="vagg")
                        state["vagg"] = vagg
                        vr0 = vagg.rearrange("p (st h u) -> p st h u",
                                             h=HPC, u=65)
                        nc.vector.memset(vr0[:, :, :, 64:65], 1.0)
                    env["QT"], env["KT"] = state["QT"], state["KT"]
                    env["VT"], env["vagg"] = state["VT"], state["vagg"]

                xt = xt_t[b, c]
                cs = slice(CHUNK * c, CHUNK * (c + 1))
                ems = []

                def qk_mm(name, w_t, b_t, dstkey):
                    def run():
                        pm = psm.tile([F, CHUNK], F32, tag="mm",
                                      name=f"pm{name}")
                        for kt in range(NKT):
                            nc.tensor.matmul(
                                pm, w_t[:, F * kt:F * (kt + 1)],
                                xt[:, CHUNK * kt:CHUNK * (kt + 1)],
                                start=(kt == 0), stop=(kt == NKT - 1),
                            )
                        rawt = rawp.tile([F, CHUNK], BF16, tag="raw",
                                         name="rawt")
                        nc.vector.tensor_scalar_add(rawt, pm, b_t[:])
                        env["raw" + name] = rawt
                    return run

                def qk_rope(name, dstkey):
                    def run():
                        rawt = env["raw" + name]
                        psw = psm.tile([F, CHUNK], F32, tag="mm", name="psw")
                        nc.tensor.matmul(psw, perm_t[:], rawt, start=True,
                                         stop=True)
                        t1 = ropet.tile([F, CHUNK], BF16, tag="t1", name="t1")
                        nc.vector.tensor_mul(t1, rawt, chat_t[:, cs])
                        t2 = ropet.tile([F, CHUNK], BF16, tag="t2", name="t2")
                        nc.vector.tensor_mul(t2, psw, shat_t[:, cs])
                        nc.vector.tensor_add(env[dstkey][:, cs], t1, t2)
                    return run

                def v_mm():
                    pmv = psm.tile([F, CHUNK], F32, tag="mm", name="pmv")
                    for kt in range(NKT):
                        nc.tensor.matmul(
                            pmv, wv_t[:, F * kt:F * (kt + 1)],
                            xt[:, CHUNK * kt:CHUNK * (kt + 1)],
                            start=(kt == 0), stop=(kt == NKT - 1),
                        )
                    nc.vector.tensor_scalar_add(env["VT"][:, cs], pmv,
                                                bv_t[:])

                def v_tr(sl0):
                    def run():
                        vr = env["vagg"].rearrange(
                            "p (st h u) -> p st h u", h=HPC, u=65)
                        for sl in (sl0, sl0 + 1):
                            st = 4 * c + sl
                            pt = psm.tile([128, 128], BF16, tag="mm",
                                          name="pt")
                            nc.tensor.transpose(
                                pt, env["VT"][:, 128 * st:128 * (st + 1)],
                                ident_t[:])
                            nc.vector.tensor_copy(
                                vr[:, st, :, 0:64],
                                pt.rearrange("p (h u) -> p h u", h=HPC),
                            )
                    return run

                def first():
                    setup()
                    qk_mm("q", wq_t, bq_t, "QT")()

                ems.append(first)
                ems.append(qk_rope("q", "QT"))
                ems.append(qk_mm("k", wk_t, bk_t, "KT"))
                ems.append(qk_rope("k", "KT"))
                ems.append(v_mm)
                ems.append(v_tr(0))
                ems.append(v_tr(2))
                return ems

            def attention_chunk(b, c, fillers=()):
                fillers = list(fillers)
                QT, KT, vagg = state["QT"], state["KT"], state["vagg"]
                vr = vagg.rearrange("p (st hu) -> p st hu", st=NST)
                sgc = sgp.tile([128, CHUNK], BF16, tag="sgc", name="sgc")
                state["sgc"] = sgc
                groups = _score_groups(c)

                def norm_act(av):
                    lnrow = recp.tile([1, CHUNK], F32, tag="lnrow",
                                      name="lnrow")
                    nc.scalar.activation(lnrow, av[64:65, :], AF.Ln)
                    recrowb = recp.tile([1, CHUNK], BF16, tag="recrowb",
                                        name="recrowb")
                    nc.scalar.activation(recrowb, lnrow, AF.Exp, scale=-1.0)
                    return recrowb

                def norm_rest(h, av, recrowb):
                    hs = slice(64 * h, 64 * (h + 1))
                    pb = psm.tile([64, CHUNK], F32, tag="mm", name="pb")
                    nc.tensor.matmul(pb, ones64_t, recrowb,
                                     start=True, stop=True)
                    recb = recp.tile([64, CHUNK], BF16, tag="recb",
                                     name="recb")
                    nc.vector.tensor_copy(recb, pb)
                    nc.vector.tensor_mul(sgc[hs, :], av[0:64, :], recb)

                avs = [psav.tile([65, CHUNK], F32, tag="av", name=f"av{h}")
                       for h in range(HPC)]
                pend = []
                for gi, grp in enumerate(groups):
                    scs = []
                    for h in range(HPC):
                        hs = slice(64 * h, 64 * (h + 1))
                        sc = pssc.tile([128, 1024], F32, tag="sc",
                                       name=f"sc{h}")
                        for kt, qlo, w, off in grp:
                            nc.tensor.matmul(
                                sc[:, off:off + w],
                                KT[hs, 128 * kt:128 * (kt + 1)],
                                QT[hs, qlo:qlo + w],
                                start=True, stop=True, skip_group_check=True,
                            )
                        scs.append(sc)
                    diag = [g2 for g2 in grp if 128 * g2[0] >= CHUNK * c]
                    if diag:
                        assert len(diag) == 2 and diag[0][3] == 0, diag
                        stride = diag[1][3]
                        m = mask_t[:]
                        mbc = bass.AP(
                            tensor=m.tensor, offset=m.offset,
                            ap=[list(m.ap[0]), [0, 2], list(m.ap[1])],
                        )
                        for sc in scs:
                            out = (sc[:, 0:2 * stride]
                                   .rearrange("p (a n) -> p a n", a=2)
                                   [:, :, 0:128])
                            nc.vector.tensor_add(out, out, mbc)
                    tot = grp[-1][3] + grp[-1][2]
                    exs = []
                    for sc in scs:
                        ex = expp.tile([128, 1024], BF16, tag="ex", name="ex")
                        nc.scalar.activation(ex[:, 0:tot], sc[:, 0:tot],
                                             AF.Exp, scale=0.125)
                        exs.append(ex)
                    if fillers:
                        fillers.pop(0)()
                    if fillers and gi % 2 == 1:
                        fillers.pop(0)()
                    if pend:
                        pgrp, pexs = pend
                        for h in range(HPC):
                            for kt, qlo, w, off in pgrp:
                                nc.tensor.matmul(
                                    avs[h][:, qlo - CHUNK * c:
                                           qlo - CHUNK * c + w],
                                    vr[:, kt, 65 * h:65 * (h + 1)],
                                    pexs[h][:, off:off + w],
                                    start=(kt == 0), stop=(kt == 4 * c + 3),
                                    skip_group_check=True,
                                )
                    pend = (grp, exs)
                pgrp, pexs = pend
                for h in range(HPC):
                    for kt, qlo, w, off in pgrp:
                        nc.tensor.matmul(
                            avs[h][:, qlo - CHUNK * c:qlo - CHUNK * c + w],
                            vr[:, kt, 65 * h:65 * (h + 1)],
                            pexs[h][:, off:off + w],
                            start=(kt == 0), stop=(kt == 4 * c + 3),
                            skip_group_check=True,
                        )
                recs = [norm_act(avs[h]) for h in range(HPC)]
                if fillers:
                    fillers.pop(0)()
                norm_rest(0, avs[0], recs[0])
                stage_half(b, c, 0)
                norm_rest(1, avs[1], recs[1])
                stage_half(b, c, 1)
                for f in fillers:
                    f()
                if debug and b == 0 and c == 0:
                    nc.scalar.dma_start(out=dbg_sg[:], in_=sgc)

            def stage_half(b, c, h):
                sgc = state["sgc"]
                hs = slice(64 * h, 64 * (h + 1))
                if b == 1 and c >= 2:
                    dst = a2a_in3[c - 2].rearrange("g (p n) -> g p n", p=F)
                    nc.scalar.dma_start(
                        out=dst[:, hs, :].rearrange("g p n -> p g n"),
                        in_=sgc[hs, :].rearrange("p (g n) -> p g n",
                                                 g=NCORES),
                    )
                else:
                    hh = 2 * b + c // 2
                    dst = a2a_in[hh].rearrange("g (p n) -> g p n", p=F)
                    nc.scalar.dma_start(
                        out=dst[4 * (c % 2):4 * (c % 2) + 4, hs, :]
                        .rearrange("g p n -> p g n"),
                        in_=sgc[hs, :].rearrange("p (g n) -> p g n", g=4),
                    )

            def fire_a2a(hh):
                nc.gpsimd.collective_compute(
                    "AllToAll", mybir.AluOpType.bypass,
                    replica_groups=[list(range(NCORES))],
                    ins=[a2a_in[hh][:]], outs=[a2a_out[hh][:]],
                )

            def fire_a2a3(j):
                nc.gpsimd.collective_compute(
                    "AllToAll", mybir.AluOpType.bypass,
                    replica_groups=[list(range(NCORES))],
                    ins=[a2a_in3[j][:]], outs=[a2a_out3[j][:]],
                )

            def outproj3(j):
                at3 = at2p.tile([128, NKT * 64], BF16, tag="at3",
                                name=f"at3{j}")
                nc.scalar.dma_start(
                    out=at3.rearrange("p (g n) -> p g n", g=NKT),
                    in_=a2a_out3[j].rearrange("g (p n) -> g p n", p=F)
                    .rearrange("g p n -> p g n"),
                )
                for eh in range(2):
                    pm = psm.tile([64, 512], F32, tag="mm", name="pyo3")
                    for kt in range(NKT):
                        nc.tensor.matmul(
                            pm, at3[:, 64 * kt:64 * (kt + 1)],
                            wot_t[:, D * kt + 512 * eh:
                                  D * kt + 512 * (eh + 1)],
                            start=(kt == 0), stop=(kt == NKT - 1),
                        )
                    ys = ysp.tile([64, 512], F32, tag="ys", name="ys3")
                    nc.vector.tensor_add(
                        ys, pm, borowb_t[0:64, 512 * eh:512 * (eh + 1)])
                    nc.scalar.dma_start(
                        out=ytq[3][64 * j:64 * (j + 1),
                                   512 * eh:512 * (eh + 1)], in_=ys)

            def outproj_emitters(hh):
                env = {}

                def load():
                    at = at2p.tile([128, NKT * 128], BF16, tag="at2",
                                   name="at2")
                    nc.scalar.dma_start(
                        out=at.rearrange("p (g n) -> p g n", g=NKT),
                        in_=a2a_out[hh].rearrange("g (p n) -> g p n", p=F)
                        .rearrange("g p n -> p g n"),
                    )
                    if debug and hh == 0:
                        nc.scalar.dma_start(out=dbg_a2ao[:],
                                            in_=a2a_out[hh][:])
                        nc.scalar.dma_start(out=dbg_a2ai[:],
                                            in_=a2a_in[hh][:])
                    env["at"] = at

                def one(eh):
                    at = env["at"]
                    pm = psm.tile([128, 512], F32, tag="mm", name="pyo")
                    for kt in range(NKT):
                        nc.tensor.matmul(
                            pm, at[:, 128 * kt:128 * (kt + 1)],
                            wot_t[:, D * kt + 512 * eh:D * kt + 512 * (eh + 1)],
                            start=(kt == 0), stop=(kt == NKT - 1),
                        )
                    ys = ysp.tile([128, 512], F32, tag="ys", name="ys")
                    nc.vector.tensor_add(
                        ys, pm, borowb_t[:, 512 * eh:512 * (eh + 1)])
                    nc.scalar.dma_start(
                        out=ytq[hh][:, 512 * eh:512 * (eh + 1)], in_=ys)

                def e0():
                    load()
                    one(0)

                return [e0, lambda: one(1)]

            # ---- main schedule (projections zippered into the previous
            # chunk's attention exp-wait bubbles) ----
            seq = [(b, c) for b in range(B) for c in range(NCH)]
            for em in proj_emitters(*seq[0]):
                em()
            for i, (b, c) in enumerate(seq):
                fillers = []
                if i + 1 < len(seq):
                    fillers += proj_emitters(*seq[i + 1])
                if (b, c) == (1, 2):
                    fillers += outproj_emitters(0)
                if (b, c) == (1, 3):
                    fillers += outproj_emitters(1) + outproj_emitters(2)
                    fillers.append(lambda: outproj3(0))
                if debug and (b, c) == (1, 0):
                    nc.scalar.dma_start(out=dbg_qt[:], in_=state["QT"][:])
                    nc.scalar.dma_start(out=dbg_kt[:], in_=state["KT"][:])
                    nc.scalar.dma_start(out=dbg_vagg[:], in_=state["vagg"][:])
                attention_chunk(b, c, fillers)
                if b == 1 and c >= 2:
                    fire_a2a3(c - 2)
                elif c % 2 == 1:
                    fire_a2a(2 * b + c // 2)
            outproj3(1)

    nc.finalize()
    return nc


_NC_CACHE = None


def _get_program():
    global _NC_CACHE
    if _NC_CACHE is None:
        _NC_CACHE = build_program()
    return _NC_CACHE


def _prep_in_maps(x, cos, sin, Wq, bq, Wk, bk, Wv, bv, Wo, bo):
    cosT = np.ascontiguousarray(cos.T).astype(np.float32)    # (32, S)
    sinT = np.ascontiguousarray(sin.T).astype(np.float32)
    chat = np.concatenate([cosT] * 4, 0).astype(BF)          # (128, S)
    shat = np.concatenate([-sinT, sinT, -sinT, sinT], 0).astype(BF)

    xtc = np.empty((B * NCH, 128, NKT * CHUNK), BF)
    for b in range(B):
        xT = np.ascontiguousarray(x[b].T).astype(np.float32)  # (1024, 2048)
        for c in range(NCH):
            blk = xT[:, CHUNK * c:CHUNK * (c + 1)]            # (1024, 512)
            xtc[NCH * b + c] = (
                blk.reshape(NKT, 128, CHUNK).transpose(1, 0, 2)
                .reshape(128, NKT * CHUNK).astype(BF)
            )

    mask128 = np.where(np.arange(128)[:, None] > np.arange(128)[None, :],
                       np.float32(MASKVAL), np.float32(0.0)).astype(np.float32)
    sw = np.arange(128)
    sw = np.where((sw // 32) % 2 == 0, sw + 32, sw - 32)
    perm128 = np.zeros((128, 128), np.float32)
    perm128[sw, np.arange(128)] = 1.0
    perm128 = perm128.astype(BF)

    woT = np.ascontiguousarray(Wo.T).astype(np.float32)       # (1024, 1024)
    wot = (woT.reshape(NKT, 128, D).transpose(1, 0, 2)
           .reshape(128, NKT * D).astype(BF))

    in_maps = []
    for core in range(NCORES):
        sl = slice(F * core, F * (core + 1))

        def wsl(W):
            wT = np.ascontiguousarray(W.T[:, sl]).astype(np.float32)
            return np.ascontiguousarray(
                wT.reshape(NKT, 128, F).transpose(1, 0, 2)
            ).reshape(128, NKT * F).astype(BF)

        in_maps.append({
            "xtc": xtc, "chat": chat, "shat": shat,
            "wq": wsl(Wq), "wk": wsl(Wk), "wv": wsl(Wv),
            "bq": np.ascontiguousarray(bq[sl]).reshape(F, 1).astype(np.float32),
            "bk": np.ascontiguousarray(bk[sl]).reshape(F, 1).astype(np.float32),
            "bv": np.ascontiguousarray(bv[sl]).reshape(F, 1).astype(np.float32),
            "ident128": np.eye(128, dtype=np.float32).astype(BF),
            "wot": wot,
            "borow": bo.reshape(1, D).astype(BF),
            "mask128": mask128, "perm128": perm128,
            "ones64": np.ones((1, 64), BF),
        })
    return in_maps


def kernel(x, cos, sin, mask, Wq, bq, Wk, bk, Wv, bv, Wo, bo, **_unused):
    """Full inputs in, full output out. `mask` (the causal mask) is
    regenerated on-device, so the input tensor itself is unused."""
    x, cos, sin = (np.asarray(a, np.float32) for a in (x, cos, sin))
    Wq, bq, Wk, bk = (np.asarray(a, np.float32) for a in (Wq, bq, Wk, bk))
    Wv, bv, Wo, bo = (np.asarray(a, np.float32) for a in (Wv, bv, Wo, bo))

    nc = _get_program()
    in_maps = _prep_in_maps(x, cos, sin, Wq, bq, Wk, bk, Wv, bv, Wo, bo)

    trace = bool(int(os.environ.get("MHA_TRACE", "0")))
    kw = {}
    if trace:
        _install_ntff_hook()
        kw = dict(trace=True, trace_cores=list(range(NCORES)))
    res = run_bass_kernel_spmd(nc, in_maps, core_ids=list(range(NCORES)), **kw)
    kernel.last_results = res

    y = np.empty((B, S, D), np.float32)
    for r in range(NCORES):
        out = res.results[r]["ytq"]          # [NH, 128, D]
        for hh in range(3):
            b, half = hh // 2, hh % 2
            base = 1024 * half + 128 * r
            y[b, base:base + 128, :] = out[hh]
        # hh==3 split per chunk: rows 0:64 = chunk2 slice, 64:128 = chunk3
        y[1, 1024 + 64 * r:1024 + 64 * (r + 1), :] = out[3][0:64]
        y[1, 1536 + 64 * r:1536 + 64 * (r + 1), :] = out[3][64:128]
    return y


# revision 36
# speedup vs baseline: 4.1971x; 4.1971x over previous
"""Multi-head attention (B=2, S=2048, D=1024, H=16, hd=64, RoPE, causal)
on 8 Trainium2 NeuronCores.

Sharding: each core owns 2 heads x both batches (tensor-parallel over heads)
through attention; the out-projection is position-sharded (each core owns a
disjoint slice of positions) via five pipelined small AllToAlls that fire as
each half-batch / chunk of attention output becomes ready, so only the last
~128KB collective is exposed at the tail.

Per core, feature-major [feature, seq] layout, all matmuls bf16:
  - inputs arrive as host-preswizzled contiguous 1MB blocks, streamed on
    BOTH HWDGE rings (sync + scalar) with the first weight/x slices split
    fine so the first matmul starts ~10us in
  - Q/K/V projections per 512-chunk; bias via DVE tensor_scalar; RoPE via a
    PE permutation matmul + 3 DVE bf16 ops; V is PE-transposed into a
    key-major aggregate with a ones-column appended per head (the softmax
    denominator falls out of the attnV matmul for free)
  - scores TRANSPOSED ST[k,q] = KT_h.T @ QT_h so no softmax max-subtraction
    is needed (scores bounded); causal at 128-tile granularity; key-tiles
    width-packed into 2-bank PSUM groups so exp runs as few wide ACT calls;
    the triangular mask is added to both diagonal tiles of a group in one
    strided DVE op
  - the two heads' score matmuls are interleaved per group: they use
    disjoint PE row groups (partitions 0-63 / 64-127), which lets the PE's
    in-silicon LDWEIGHTS pull-ahead overlap weight loads with matmuls (the
    walrus build here has ldw double-buffering disabled, so this is the only
    way to dodge the ~350-cycle per-matmul LDW+drain tax)
  - softmax normalization: denominator reciprocal via ACT Ln/Exp (same
    table set as the score exp), broadcast across partitions by a rank-1 PE
    matmul, applied by DVE
  - next-chunk projection matmuls are zippered into the current chunk's
    attention instruction stream (one emitter per exp group), filling the
    PE bubbles where attnV waits on ACT; the final out-projections ride the
    last attention chunk the same way
  - out-projection consumes the AllToAll'd [1024 feat x pos] slabs and
    produces y[pos, D] directly (positions on partitions), bias added by
    DVE from a partition-broadcast bias tile
"""
import os

import ml_dtypes
import numpy as np

import concourse.bass as bass
import concourse.mybir as mybir
import concourse.tile as tile
from concourse.bass_utils import run_bass_kernel_spmd
from concourse.vector_clock import ScopedClock

B, S, D, H, HD = 2, 2048, 1024, 16, 64
NCORES = 8
HPC = 2                    # heads per core
F = HPC * HD               # 128 features per core
CHUNK = 512
NCH = S // CHUNK           # 4 q-chunks per batch
NKT = D // 128             # 8 contraction tiles for projections
NST = S // 128             # 16 key tiles
NH = 4                     # half-batches (b, half) = collective units
MASKVAL = -240.0           # -30 after the 1/8 softmax scale
F32 = mybir.dt.float32
F32R = mybir.dt.float32r
BF16 = mybir.dt.bfloat16
BF = ml_dtypes.bfloat16
AF = mybir.ActivationFunctionType


# ---------------------------------------------------------------------------
# Workarounds for the walrus build in this container: it encodes at most ONE
# sync-wait per instruction ("Too many sync wait commands"). Split multi-wait
# instructions into single-wait NoOps. Semantics-preserving.
# ---------------------------------------------------------------------------
_patched = False


def _install_patches():
    global _patched
    if _patched:
        return
    _patched = True

    _orig_lower = tile.TileContext._lower_ordered_insts

    def _lower_with_wait_split(self, ordered):
        nc = self.nc
        for _bb, insts in ordered.items():
            if not any(
                i.sync_info is not None and len(i.sync_info.on_wait) > 1
                for i in insts
            ):
                continue
            new = []
            for inst in insts:
                si = inst.sync_info
                if si is not None and len(si.on_wait) > 1:
                    waits = list(si.on_wait)
                    for w in waits[:-1]:
                        n = mybir.InstNoOp(
                            name=f"I-waitsplit-{nc.next_id()}", ins=[], outs=[]
                        )
                        n.engine = inst.engine
                        n.bass_nofuse = True
                        n.sync_info = mybir.SyncInfo(on_wait=[w], on_update=[])
                        nc.register_instruction(n)
                        new.append(n)
                    inst.sync_info = mybir.SyncInfo(
                        on_wait=[waits[-1]], on_update=list(si.on_update)
                    )
                new.append(inst)
            insts[:] = new
        return _orig_lower(self, ordered)

    tile.TileContext._lower_ordered_insts = _lower_with_wait_split

    def _drain_and_barrier(self, tick_clock, wait_clock):
        nc = self.nc
        probe = nc.sync.nop(nofuse=True)
        wait_clock.add_sem_waits(
            probe.ins, ScopedClock({None: tick_clock.global_clock})
        )
        waits = list(probe.ins.sync_info.on_wait)
        probe.ins.sync_info = mybir.SyncInfo(on_wait=waits[:1], on_update=[])
        for w in waits[1:]:
            n2 = nc.sync.nop(nofuse=True)
            n2.ins.sync_info = mybir.SyncInfo(on_wait=[w], on_update=[])
        nc.sync.drain()
        nc.all_engine_barrier()
        assert self.sems is not None
        popped = nc._tile_sem_poison_stack.pop()
        assert popped is self._sem_poison
        nc.clear_and_free_semaphores(list(self.sems.allocated().values()))
        nc.all_engine_barrier()

    tile.TileContext._drain_and_barrier = _drain_and_barrier


def _install_ntff_hook():
    """Provide the missing ``antenv.axon_hooks`` module so trace=True works."""
    import sys
    import types

    if "antenv.axon_hooks" in sys.modules:
        return
    try:
        import antenv
        from trn_agent_boot.trn_boot import _ntff_profile_via_ctypes
    except ImportError:
        return
    mod = types.ModuleType("antenv.axon_hooks")
    mod._hook = _ntff_profile_via_ctypes("/opt/axon/libaxon_pjrt.so")
    mod.set_axon_ntff_profile_hook = lambda h: setattr(mod, "_hook", h)
    mod.get_axon_ntff_profile_hook = lambda: mod._hook
    sys.modules["antenv.axon_hooks"] = mod
    antenv.axon_hooks = mod


def _score_groups(c):
    """Pack the causal key-tiles of q-chunk c into PSUM groups of <=1024
    columns. Returns [[(kt, qlo, w, off), ...], ...]."""
    groups, cur, cw = [], [], 0
    for kt in range(4 * c + 4):
        qlo = max(CHUNK * c, 128 * kt)
        w = CHUNK * (c + 1) - qlo
        if cw + w > 1024:
            groups.append(cur)
            cur, cw = [], 0
        # matmul output must not straddle a PSUM bank boundary
        assert cw % 512 == 0 or cw + w <= 512, (c, kt, cw, w)
        cur.append((kt, qlo, w, cw))
        cw += w
    if cur:
        groups.append(cur)
    return groups


# ---------------------------------------------------------------------------
# Program builder (same program on all 8 cores; per-core data differs)
# ---------------------------------------------------------------------------
def build_program():
    _install_patches()
    nc = bass.Bass(num_devices=NCORES)

    xtc_d = nc.dram_tensor("xtc", [B * NCH, 128, NKT * CHUNK], BF16,
                           kind="ExternalInput")
    wq_d = nc.dram_tensor("wq", [128, NKT * F], BF16, kind="ExternalInput")
    wk_d = nc.dram_tensor("wk", [128, NKT * F], BF16, kind="ExternalInput")
    wv_d = nc.dram_tensor("wv", [128, NKT * F], BF16, kind="ExternalInput")
    bq_d = nc.dram_tensor("bq", [F, 1], F32, kind="ExternalInput")
    bk_d = nc.dram_tensor("bk", [F, 1], F32, kind="ExternalInput")
    bv_d = nc.dram_tensor("bv", [F, 1], F32, kind="ExternalInput")
    ident_d = nc.dram_tensor("ident128", [128, 128], BF16,
                             kind="ExternalInput")
    chat_d = nc.dram_tensor("chat", [128, S], BF16, kind="ExternalInput")
    shat_d = nc.dram_tensor("shat", [128, S], BF16, kind="ExternalInput")
    mask_d = nc.dram_tensor("mask128", [128, 128], F32, kind="ExternalInput")
    perm_d = nc.dram_tensor("perm128", [128, 128], BF16, kind="ExternalInput")
    ones64_d = nc.dram_tensor("ones64", [1, 64], BF16, kind="ExternalInput")
    borow_d = nc.dram_tensor("borow", [1, D], BF16, kind="ExternalInput")
    wot_d = nc.dram_tensor("wot", [128, NKT * D], BF16, kind="ExternalInput")
    ytq = nc.dram_tensor("ytq", [NH, 128, D], F32, kind="ExternalOutput")

    debug = bool(int(os.environ.get("MHA_DEBUG", "0")))
    if debug:
        dbg_qt = nc.dram_tensor("dbg_qt", [F, S], BF16, kind="ExternalOutput")
        dbg_kt = nc.dram_tensor("dbg_kt", [F, S], BF16, kind="ExternalOutput")
        dbg_vagg = nc.dram_tensor("dbg_vagg", [128, NST * HPC * 65], BF16,
                                  kind="ExternalOutput")
        dbg_sg = nc.dram_tensor("dbg_sg", [128, CHUNK], BF16,
                                kind="ExternalOutput")
        dbg_a2ao = nc.dram_tensor("dbg_a2ao", [NCORES, F * 128], BF16,
                                  kind="ExternalOutput")
        dbg_a2ai = nc.dram_tensor("dbg_a2ai", [NCORES, F * 128], BF16,
                                  kind="ExternalOutput")

    a2a_in = [nc.dram_tensor(f"a2ain{hh}", [NCORES, F * 128], BF16)
              for hh in range(3)]
    a2a_out = [nc.dram_tensor(f"a2aout{hh}", [NCORES, F * 128], BF16)
               for hh in range(3)]
    a2a_in3 = [nc.dram_tensor(f"a2ain3{j}", [NCORES, F * 64], BF16)
               for j in range(2)]
    a2a_out3 = [nc.dram_tensor(f"a2aout3{j}", [NCORES, F * 64], BF16)
                for j in range(2)]

    with tile.TileContext(nc) as tc:
        with (
            tc.tile_pool(name="const", bufs=1) as const,
            tc.tile_pool(name="xtp", bufs=1) as xtp,
            tc.tile_pool(name="qkv", bufs=2) as qkv,
            tc.tile_pool(name="vaggp", bufs=2) as vaggp,
            tc.tile_pool(name="rawp", bufs=2) as rawp,
            tc.tile_pool(name="ropet", bufs=2) as ropet,
            tc.tile_pool(name="exp", bufs=4) as expp,
            tc.tile_pool(name="recp", bufs=2) as recp,
            tc.tile_pool(name="sgp", bufs=2) as sgp,
            tc.tile_pool(name="at2p", bufs=2) as at2p,
            tc.tile_pool(name="ysp", bufs=2) as ysp,
            tc.tile_pool(name="psm", bufs=2, space="PSUM") as psm,
            tc.tile_pool(name="pssc", bufs=2, space="PSUM") as pssc,
            tc.tile_pool(name="psav", bufs=2, space="PSUM") as psav,
        ):
            # ---- input DMAs, priority order on the sync ring ----
            xt_t = {}
            # ramp-critical path on the sync ring: wq + first x chunk only;
            # everything else for chunk 0/1 goes to the scalar ring so both
            # HWDGE rings stream in parallel.
            wq_t = const.tile([128, NKT * F], BF16, name="wq")
            nc.sync.dma_start(out=wq_t[:, 0:F], in_=wq_d[:, 0:F])
            bq_t = const.tile([F, 1], F32, name="bq")
            nc.sync.dma_start(out=bq_t, in_=bq_d[:])
            t00 = xtp.tile([128, NKT * CHUNK], BF16, tag="xt00", name="xt00")
            nc.sync.dma_start(out=t00[:, 0:CHUNK], in_=xtc_d[0][:, 0:CHUNK])
            nc.sync.dma_start(out=wq_t[:, F:2 * F], in_=wq_d[:, F:2 * F])
            nc.sync.dma_start(out=t00[:, CHUNK:2 * CHUNK],
                              in_=xtc_d[0][:, CHUNK:2 * CHUNK])
            nc.sync.dma_start(out=wq_t[:, 2 * F:], in_=wq_d[:, 2 * F:])
            nc.sync.dma_start(out=t00[:, 2 * CHUNK:4 * CHUNK],
                              in_=xtc_d[0][:, 2 * CHUNK:4 * CHUNK])
            nc.sync.dma_start(out=t00[:, 4 * CHUNK:],
                              in_=xtc_d[0][:, 4 * CHUNK:])
            xt_t[0, 0] = t00
            wk_t = const.tile([128, NKT * F], BF16, name="wk")
            wv_t = const.tile([128, NKT * F], BF16, name="wv")
            nc.scalar.dma_start(out=wk_t, in_=wk_d[:])
            nc.scalar.dma_start(out=wv_t, in_=wv_d[:])
            bk_t = const.tile([F, 1], F32, name="bk")
            bv_t = const.tile([F, 1], F32, name="bv")
            ident_t = const.tile([128, 128], BF16, name="ident")
            nc.scalar.dma_start(out=bk_t, in_=bk_d[:])
            nc.scalar.dma_start(out=bv_t, in_=bv_d[:])
            nc.scalar.dma_start(out=ident_t, in_=ident_d[:])
            chat_t = const.tile([128, S], BF16, name="chat")
            shat_t = const.tile([128, S], BF16, name="shat")
            nc.scalar.dma_start(out=chat_t, in_=chat_d[:])
            nc.scalar.dma_start(out=shat_t, in_=shat_d[:])
            mask_t = const.tile([128, 128], F32, name="mask")
            perm_t = const.tile([128, 128], BF16, name="perm")
            ones64_t = const.tile([1, 64], BF16, name="ones64")
            borowb_t = const.tile([128, D], BF16, name="borowb")
            nc.scalar.dma_start(out=mask_t, in_=mask_d[:])
            nc.scalar.dma_start(out=perm_t, in_=perm_d[:])
            nc.scalar.dma_start(out=ones64_t, in_=ones64_d[:])
            _br = borow_d[:]
            nc.scalar.dma_start(
                out=borowb_t,
                in_=bass.AP(tensor=_br.tensor, offset=_br.offset,
                            ap=[[0, 128]] + [list(p) for p in _br.ap[1:]]),
            )
            t01 = xtp.tile([128, NKT * CHUNK], BF16, tag="xt01", name="xt01")
            nc.scalar.dma_start(out=t01, in_=xtc_d[1])
            xt_t[0, 1] = t01
            for b in range(B):
                for c in range(NCH):
                    if (b, c) in ((0, 0), (0, 1)):
                        continue
                    t = xtp.tile([128, NKT * CHUNK], BF16, tag=f"xt{b}{c}",
                                 name=f"xt{b}{c}")
                    nc.sync.dma_start(out=t, in_=xtc_d[NCH * b + c])
                    xt_t[b, c] = t
            # out-proj weights on the scalar ring (idle early)
            wot_t = const.tile([128, NKT * D], BF16, name="wot")
            nc.scalar.dma_start(out=wot_t, in_=wot_d[:])

            state = {}

            def proj_emitters(b, c):
                """Fine-grained emitter closures for chunk (b, c)'s
                projections, to zipper into the previous chunk's attention."""
                env = {}

                def setup():
                    if c == 0:
                        state["QT"] = qkv.tile([F, S], BF16, tag="QT",
                                               name="QT")
                        state["KT"] = qkv.tile([F, S], BF16, tag="KT",
                                               name="KT")
                        state["VT"] = qkv.tile([F, S], BF16, tag="VT",
                                               name="VT")
                        vagg = vaggp.tile([128, NST * HPC * 65], BF16,
                                          tag="vagg", name="vagg")
                        state["vagg"] = vagg
                        vr0 = vagg.rearrange("p (st h u) -> p st h u",
                                             h=HPC, u=65)
                        nc.vector.memset(vr0[:, :, :, 64:65], 1.0)
                    env["QT"], env["KT"] = state["QT"], state["KT"]
                    env["VT"], env["vagg"] = state["VT"], state["vagg"]

                xt = xt_t[b, c]
                cs = slice(CHUNK * c, CHUNK * (c + 1))
                ems = []

                def qk_mm(name, w_t, b_t, dstkey):
                    def run():
                        pm = psm.tile([F, CHUNK], F32, tag="mm",
                                      name=f"pm{name}")
                        for kt in range(NKT):
                            nc.tensor.matmul(
                                pm, w_t[:, F * kt:F * (kt + 1)],
                                xt[:, CHUNK * kt:CHUNK * (kt + 1)],
                                start=(kt == 0), stop=(kt == NKT - 1),
                            )
                        rawt = rawp.tile([F, CHUNK], BF16, tag="raw",
                                         name="rawt")
                        nc.vector.tensor_scalar_add(rawt, pm, b_t[:])
                        env["raw" + name] = rawt
                    return run

                def qk_rope(name, dstkey):
                    def run():
                        rawt = env["raw" + name]
                        psw = psm.tile([F, CHUNK], F32, tag="mm", name="psw")
                        nc.tensor.matmul(psw, perm_t[:], rawt, start=True,
                                         stop=True)
                        t1 = ropet.tile([F, CHUNK], BF16, tag="t1", name="t1")
                        nc.vector.tensor_mul(t1, rawt, chat_t[:, cs])
                        t2 = ropet.tile([F, CHUNK], BF16, tag="t2", name="t2")
                        nc.vector.tensor_mul(t2, psw, shat_t[:, cs])
                        nc.vector.tensor_add(env[dstkey][:, cs], t1, t2)
                    return run

                def v_mm():
                    pmv = psm.tile([F, CHUNK], F32, tag="mm", name="pmv")
                    for kt in range(NKT):
                        nc.tensor.matmul(
                            pmv, wv_t[:, F * kt:F * (kt + 1)],
                            xt[:, CHUNK * kt:CHUNK * (kt + 1)],
                            start=(kt == 0), stop=(kt == NKT - 1),
                        )
                    nc.vector.tensor_scalar_add(env["VT"][:, cs], pmv,
                                                bv_t[:])

                def v_tr(sl0):
                    def run():
                        vr = env["vagg"].rearrange(
                            "p (st h u) -> p st h u", h=HPC, u=65)
                        for sl in (sl0, sl0 + 1):
                            st = 4 * c + sl
                            pt = psm.tile([128, 128], BF16, tag="mm",
                                          name="pt")
                            nc.tensor.transpose(
                                pt, env["VT"][:, 128 * st:128 * (st + 1)],
                                ident_t[:])
                            nc.vector.tensor_copy(
                                vr[:, st, :, 0:64],
                                pt.rearrange("p (h u) -> p h u", h=HPC),
                            )
                    return run

                def first():
                    setup()
                    qk_mm("q", wq_t, bq_t, "QT")()

                ems.append(first)
                ems.append(qk_rope("q", "QT"))
                ems.append(qk_mm("k", wk_t, bk_t, "KT"))
                ems.append(qk_rope("k", "KT"))
                ems.append(v_mm)
                ems.append(v_tr(0))
                ems.append(v_tr(2))
                return ems

            def attention_chunk(b, c, fillers=()):
                fillers = list(fillers)
                QT, KT, vagg = state["QT"], state["KT"], state["vagg"]
                vr = vagg.rearrange("p (st hu) -> p st hu", st=NST)
                sgc = sgp.tile([128, CHUNK], BF16, tag="sgc", name="sgc")
                state["sgc"] = sgc
                groups = _score_groups(c)

                def norm_act(av):
                    lnrow = recp.tile([1, CHUNK], F32, tag="lnrow",
                                      name="lnrow")
                    nc.scalar.activation(lnrow, av[64:65, :], AF.Ln)
                    recrowb = recp.tile([1, CHUNK], BF16, tag="recrowb",
                                        name="recrowb")
                    nc.scalar.activation(recrowb, lnrow, AF.Exp, scale=-1.0)
                    return recrowb

                def norm_rest(h, av, recrowb):
                    hs = slice(64 * h, 64 * (h + 1))
                    pb = psm.tile([64, CHUNK], F32, tag="mm", name="pb")
                    nc.tensor.matmul(pb, ones64_t, recrowb,
                                     start=True, stop=True)
                    recb = recp.tile([64, CHUNK], BF16, tag="recb",
                                     name="recb")
                    nc.vector.tensor_copy(recb, pb)
                    nc.vector.tensor_mul(sgc[hs, :], av[0:64, :], recb)

                avs = [psav.tile([65, CHUNK], F32, tag="av", name=f"av{h}")
                       for h in range(HPC)]
                pend = []
                for gi, grp in enumerate(groups):
                    scs = []
                    for h in range(HPC):
                        hs = slice(64 * h, 64 * (h + 1))
                        sc = pssc.tile([128, 1024], F32, tag="sc",
                                       name=f"sc{h}")
                        for kt, qlo, w, off in grp:
                            nc.tensor.matmul(
                                sc[:, off:off + w],
                                KT[hs, 128 * kt:128 * (kt + 1)],
                                QT[hs, qlo:qlo + w],
                                start=True, stop=True, skip_group_check=True,
                            )
                        scs.append(sc)
                    diag = [g2 for g2 in grp if 128 * g2[0] >= CHUNK * c]
                    if diag:
                        assert len(diag) == 2 and diag[0][3] == 0, diag
                        stride = diag[1][3]
                        m = mask_t[:]
                        mbc = bass.AP(
                            tensor=m.tensor, offset=m.offset,
                            ap=[list(m.ap[0]), [0, 2], list(m.ap[1])],
                        )
                        for sc in scs:
                            out = (sc[:, 0:2 * stride]
                                   .rearrange("p (a n) -> p a n", a=2)
                                   [:, :, 0:128])
                            nc.vector.tensor_add(out, out, mbc)
                    tot = grp[-1][3] + grp[-1][2]
                    exs = []
                    for sc in scs:
                        ex = expp.tile([128, 1024], BF16, tag="ex", name="ex")
                        nc.scalar.activation(ex[:, 0:tot], sc[:, 0:tot],
                                             AF.Exp, scale=0.125)
                        exs.append(ex)
                    if fillers:
                        fillers.pop(0)()
                    if fillers and gi % 2 == 1:
                        fillers.pop(0)()
                    if pend:
                        pgrp, pexs = pend
                        for h in range(HPC):
                            for kt, qlo, w, off in pgrp:
                                nc.tensor.matmul(
                                    avs[h][:, qlo - CHUNK * c:
                                           qlo - CHUNK * c + w],
                                    vr[:, kt, 65 * h:65 * (h + 1)],
                                    pexs[h][:, off:off + w],
                                    start=(kt == 0), stop=(kt == 4 * c + 3),
                                    skip_group_check=True,
                                )
                    pend = (grp, exs)
                pgrp, pexs = pend
                for h in range(HPC):
                    for kt, qlo, w, off in pgrp:
                        nc.tensor.matmul(
                            avs[h][:, qlo - CHUNK * c:qlo - CHUNK * c + w],
                            vr[:, kt, 65 * h:65 * (h + 1)],
                            pexs[h][:, off:off + w],
                            start=(kt == 0), stop=(kt == 4 * c + 3),
                            skip_group_check=True,
                        )
                recs = [norm_act(avs[h]) for h in range(HPC)]
                if fillers:
                    fillers.pop(0)()
                norm_rest(0, avs[0], recs[0])
                stage_half(b, c, 0)
                norm_rest(1, avs[1], recs[1])
                stage_half(b, c, 1)
                for f in fillers:
                    f()
                if debug and b == 0 and c == 0:
                    nc.scalar.dma_start(out=dbg_sg[:], in_=sgc)

            def stage_half(b, c, h):
                sgc = state["sgc"]
                hs = slice(64 * h, 64 * (h + 1))
                if b == 1 and c >= 2:
                    dst = a2a_in3[c - 2].rearrange("g (p n) -> g p n", p=F)
                    nc.scalar.dma_start(
                        out=dst[:, hs, :].rearrange("g p n -> p g n"),
                        in_=sgc[hs, :].rearrange("p (g n) -> p g n",
                                                 g=NCORES),
                    )
                else:
                    hh = 2 * b + c // 2
                    dst = a2a_in[hh].rearrange("g (p n) -> g p n", p=F)
                    nc.scalar.dma_start(
                        out=dst[4 * (c % 2):4 * (c % 2) + 4, hs, :]
                        .rearrange("g p n -> p g n"),
                        in_=sgc[hs, :].rearrange("p (g n) -> p g n", g=4),
                    )

            def fire_a2a(hh):
                nc.gpsimd.collective_compute(
                    "AllToAll", mybir.AluOpType.bypass,
                    replica_groups=[list(range(NCORES))],
                    ins=[a2a_in[hh][:]], outs=[a2a_out[hh][:]],
                )

            def fire_a2a3(j):
                nc.gpsimd.collective_compute(
                    "AllToAll", mybir.AluOpType.bypass,
                    replica_groups=[list(range(NCORES))],
                    ins=[a2a_in3[j][:]], outs=[a2a_out3[j][:]],
                )

            def outproj3(j):
                at3 = at2p.tile([128, NKT * 64], BF16, tag="at3",
                                name=f"at3{j}")
                nc.scalar.dma_start(
                    out=at3.rearrange("p (g n) -> p g n", g=NKT),
                    in_=a2a_out3[j].rearrange("g (p n) -> g p n", p=F)
                    .rearrange("g p n -> p g n"),
                )
                for eh in range(2):
                    pm = psm.tile([64, 512], F32, tag="mm", name="pyo3")
                    for kt in range(NKT):
                        nc.tensor.matmul(
                            pm, at3[:, 64 * kt:64 * (kt + 1)],
                            wot_t[:, D * kt + 512 * eh:
                                  D * kt + 512 * (eh + 1)],
                            start=(kt == 0), stop=(kt == NKT - 1),
                        )
                    ys = ysp.tile([64, 512], F32, tag="ys", name="ys3")
                    nc.vector.tensor_add(
                        ys, pm, borowb_t[0:64, 512 * eh:512 * (eh + 1)])
                    nc.scalar.dma_start(
                        out=ytq[3][64 * j:64 * (j + 1),
                                   512 * eh:512 * (eh + 1)], in_=ys)

            def outproj_emitters(hh):
                env = {}

                def load():
                    at = at2p.tile([128, NKT * 128], BF16, tag="at2",
                                   name="at2")
                    nc.scalar.dma_start(
                        out=at.rearrange("p (g n) -> p g n", g=NKT),
                        in_=a2a_out[hh].rearrange("g (p n) -> g p n", p=F)
                        .rearrange("g p n -> p g n"),
                    )
                    if debug and hh == 0:
                        nc.scalar.dma_start(out=dbg_a2ao[:],
                                            in_=a2a_out[hh][:])
                        nc.scalar.dma_start(out=dbg_a2ai[:],
                                            in_=a2a_in[hh][:])
                    env["at"] = at

                def one(eh):
                    at = env["at"]
                    pm = psm.tile([128, 512], F32, tag="mm", name="pyo")
                    for kt in range(NKT):
                        nc.tensor.matmul(
                            pm, at[:, 128 * kt:128 * (kt + 1)],
                            wot_t[:, D * kt + 512 * eh:D * kt + 512 * (eh + 1)],
                            start=(kt == 0), stop=(kt == NKT - 1),
                        )
                    ys = ysp.tile([128, 512], F32, tag="ys", name="ys")
                    nc.vector.tensor_add(
                        ys, pm, borowb_t[:, 512 * eh:512 * (eh + 1)])
                    nc.scalar.dma_start(
                        out=ytq[hh][:, 512 * eh:512 * (eh + 1)], in_=ys)

                def e0():
                    load()
                    one(0)

                return [e0, lambda: one(1)]

            # ---- main schedule (projections zippered into the previous
            # chunk's attention exp-wait bubbles) ----
            seq = [(b, c) for b in range(B) for c in range(NCH)]
            for em in proj_emitters(*seq[0]):
                em()
            for i, (b, c) in enumerate(seq):
                fillers = []
                if i + 1 < len(seq):
                    fillers += proj_emitters(*seq[i + 1])
                if (b, c) == (1, 2):
                    fillers += outproj_emitters(0)
                if (b, c) == (1, 3):
                    fillers += outproj_emitters(1) + outproj_emitters(2)
                    fillers.append(lambda: outproj3(0))
                if debug and (b, c) == (1, 0):
                    nc.scalar.dma_start(out=dbg_qt[:], in_=state["QT"][:])
                    nc.scalar.dma_start(out=dbg_kt[:], in_=state["KT"][:])
                    nc.scalar.dma_start(out=dbg_vagg[:], in_=state["vagg"][:])
                attention_chunk(b, c, fillers)
                if b == 1 and c >= 2:
                    fire_a2a3(c - 2)
                elif c % 2 == 1:
                    fire_a2a(2 * b + c // 2)
            outproj3(1)

    nc.finalize()
    return nc


_NC_CACHE = None


def _get_program():
    global _NC_CACHE
    if _NC_CACHE is None:
        _NC_CACHE = build_program()
    return _NC_CACHE


def _prep_in_maps(x, cos, sin, Wq, bq, Wk, bk, Wv, bv, Wo, bo):
    cosT = np.ascontiguousarray(cos.T).astype(np.float32)    # (32, S)
    sinT = np.ascontiguousarray(sin.T).astype(np.float32)
    chat = np.concatenate([cosT] * 4, 0).astype(BF)          # (128, S)
    shat = np.concatenate([-sinT, sinT, -sinT, sinT], 0).astype(BF)

    xtc = np.empty((B * NCH, 128, NKT * CHUNK), BF)
    for b in range(B):
        xT = np.ascontiguousarray(x[b].T).astype(np.float32)  # (1024, 2048)
        for c in range(NCH):
            blk = xT[:, CHUNK * c:CHUNK * (c + 1)]            # (1024, 512)
            xtc[NCH * b + c] = (
                blk.reshape(NKT, 128, CHUNK).transpose(1, 0, 2)
                .reshape(128, NKT * CHUNK).astype(BF)
            )

    mask128 = np.where(np.arange(128)[:, None] > np.arange(128)[None, :],
                       np.float32(MASKVAL), np.float32(0.0)).astype(np.float32)
    sw = np.arange(128)
    sw = np.where((sw // 32) % 2 == 0, sw + 32, sw - 32)
    perm128 = np.zeros((128, 128), np.float32)
    perm128[sw, np.arange(128)] = 1.0
    perm128 = perm128.astype(BF)

    woT = np.ascontiguousarray(Wo.T).astype(np.float32)       # (1024, 1024)
    wot = (woT.reshape(NKT, 128, D).transpose(1, 0, 2)
           .reshape(128, NKT * D).astype(BF))

    in_maps = []
    for core in range(NCORES):
        sl = slice(F * core, F * (core + 1))

        def wsl(W):
            wT = np.ascontiguousarray(W.T[:, sl]).astype(np.float32)
            return np.ascontiguousarray(
                wT.reshape(NKT, 128, F).transpose(1, 0, 2)
            ).reshape(128, NKT * F).astype(BF)

        in_maps.append({
            "xtc": xtc, "chat": chat, "shat": shat,
            "wq": wsl(Wq), "wk": wsl(Wk), "wv": wsl(Wv),
            "bq": np.ascontiguousarray(bq[sl]).reshape(F, 1).astype(np.float32),
            "bk": np.ascontiguousarray(bk[sl]).reshape(F, 1).astype(np.float32),
            "bv": np.ascontiguousarray(bv[sl]).reshape(F, 1).astype(np.float32),
            "ident128": np.eye(128, dtype=np.float32).astype(BF),
            "wot": wot,
            "borow": bo.reshape(1, D).astype(BF),
            "mask128": mask128, "perm128": perm128,
            "ones64": np.ones((1, 64), BF),
        })
    return in_maps


def kernel(x, cos, sin, mask, Wq, bq, Wk, bk, Wv, bv, Wo, bo, **_unused):
    """Full inputs in, full output out. `mask` (the causal mask) is
    regenerated on-device, so the input tensor itself is unused."""
    x, cos, sin = (np.asarray(a, np.float32) for a in (x, cos, sin))
    Wq, bq, Wk, bk = (np.asarray(a, np.float32) for a in (Wq, bq, Wk, bk))
    Wv, bv, Wo, bo = (np.asarray(a, np.float32) for a in (Wv, bv, Wo, bo))

    nc = _get_program()
    in_maps = _prep_in_maps(x, cos, sin, Wq, bq, Wk, bk, Wv, bv, Wo, bo)

    trace = bool(int(os.environ.get("MHA_TRACE", "0")))
    kw = {}
    if trace:
        _install_ntff_hook()
        kw = dict(trace=True, trace_cores=list(range(NCORES)))
    res = run_bass_kernel_spmd(nc, in_maps, core_ids=list(range(NCORES)), **kw)
    kernel.last_results = res

    y = np.empty((B, S, D), np.float32)
    for r in range(NCORES):
        out = res.results[r]["ytq"]          # [NH, 128, D]
        for hh in range(3):
            b, half = hh // 2, hh % 2
            base = 1024 * half + 128 * r
            y[b, base:base + 128, :] = out[hh]
        # hh==3 split per chunk: rows 0:64 = chunk2 slice, 64:128 = chunk3
        y[1, 1024 + 64 * r:1024 + 64 * (r + 1), :] = out[3][0:64]
        y[1, 1536 + 64 * r:1536 + 64 * (r + 1), :] = out[3][64:128]
    return y
